# revision 21
# baseline (speedup 1.0000x reference)
"""Trainium2 Bass kernel for nn_Brain_17789754540385.

Model: 4 stacked Keras LSTMs (units=3) over (B=8192, T=256) scalar sequences,
then Dense(3->1); output (1, B).

Strategy (pure data parallel, 8 cores, 1024 batch rows each):
- Batch-on-partitions layout: 1024 rows = 8 groups x 128 partitions per
  core, split into 3 independent superstreams of (3, 3, 2) groups. Each
  stream is a fully serial recurrence chain (~2.5us per wavefront step);
  the wall clock is max(chain latency, engine throughput), and 3 streams
  is the measured optimum: fewer streams leave engines idle inside the
  chain latency, more streams saturate the DVE with per-op fixed costs.
- Wavefront over s = 0..258: layer l computes timestep t = s - l. Biases
  are zero, so zero state is a fixed point and wavefront edges need no
  masking (only edge slots are zeroed; interior slots are written before
  read).
- State representation: each slot stores (x, P_l = to_l*tc_l, tc_l) per
  layer, since 2h = (to+1)*tc = P + tc; both P and tc rows carry half the
  h-weights in the combined matmul, so no (to+1) op is materialized.
- Per superstream, per step (one serial chain):
    PE : transpose the fp16 state slot (128, 25g) -> (25g, 128) PSUM
    VEC: copy PSUM -> SBUF (matmul stationary must be SBUF)
    PE : block-diagonal matmul vs combined weights -> z (128, 48g) f32
    ACT: GD[tg|tf|ti|to] = tanh(z)   (sigmoid folded: sig(x) =
         0.5 + 0.5*tanh(x/2), the 1/2 lives in the f,i,o weight columns)
    VEC: [Q|R] = ([tf|ti]+1) * [D|tg]  (one fused op; D stores 2c and
         sits in the same tile as the gates to make the operands
         contiguous; Q = 4fc, R = 2ig)
    VEC: D' = 0.5*Q + R               (= 2c', in place)
    ACT: tc = tanh(0.5*D) -> next slot
    VEC: P = to * tc      -> next slot
- All 16-bit tensors are fp16 (better mantissa than bf16; rel err 1.9e-3
  vs 3.9e-3). GPSIMD only does setup work: per-step ops on the Q7 are
  280-870ns and poison the chain.
- Final dense (3 -> 1) and batch gather run on host in fp32.
"""

import numpy as np
import ml_dtypes

F16 = np.float16

UNITS = 3
N_CORES = 8
B = 8192
T = 256
NG = 8            # batch groups of 128 per core
NSTREAM = 3       # superstreams
GCS = [3, 3, 2]   # groups per superstream
GOFF = [0, 3, 6]
SLOT = 25         # comps per group per slot: [x, P1..P4 (12), tc1..tc4 (12)]
SWS = [g * SLOT for g in GCS]   # slot width per superstream
ZWS = [g * 48 for g in GCS]     # gate width per superstream
CWS = [g * 12 for g in GCS]     # cell width per superstream
NSTEP = T + 3     # 259 wavefront steps
NSLOT = NSTEP + 1

_BUILT = {}


# ---------------------------------------------------------------------------
# host-side weight prep
# ---------------------------------------------------------------------------

def _build_wcomb(w, u):
    """Combined stationary-side weight matrix (25, 48) as fp64->bf16.

    Rows: 0 = x; 1+3l+v = P_l unit v; 13+3l+v = tc_l unit v.
    Cols: gate-major a-blocks of 12: [g, f, i, o] x (layer-major (l, v)).
    Keras gate order in w/u is i, f, g, o. h_l = 0.5*(P_l + tc_l), so both
    P_l and tc_l rows carry half the h_l weights. f,i,o columns are halved
    once more for the sigmoid-as-tanh fold.
    """
    kmap = [2, 1, 0, 3]  # target gate a=[g,f,i,o] -> keras col block
    wcomb = np.zeros((SLOT, 48), np.float64)
    for l in range(4):
        wl = np.asarray(w[l], np.float64)   # (in_dim, 12) keras order
        ul = np.asarray(u[l], np.float64)   # (3, 12)
        for a in range(4):
            gs = 1.0 if a == 0 else 0.5
            for v in range(3):
                col = a * 12 + l * 3 + v
                kc = kmap[a] * 3 + v
                # recurrent: h_l rows (P_l and tc_l, half weight each)
                for vv in range(3):
                    wcomb[1 + 3 * l + vv, col] = ul[vv, kc] * gs * 0.5
                    wcomb[13 + 3 * l + vv, col] = ul[vv, kc] * gs * 0.5
                # input: x for layer 0, h_{l-1} rows otherwise
                if l == 0:
                    wcomb[0, col] = wl[0, kc] * gs
                else:
                    for vv in range(3):
                        wcomb[1 + 3 * (l - 1) + vv, col] = (
                            wl[vv, kc] * gs * 0.5
                        )
                        wcomb[13 + 3 * (l - 1) + vv, col] = (
                            wl[vv, kc] * gs * 0.5
                        )
    return wcomb.astype(F16)


# ---------------------------------------------------------------------------
# workarounds: this walrus build allows at most ONE sem wait per instruction
# ---------------------------------------------------------------------------

def _install_patches():
    import concourse.tile as tile_mod
    from concourse import mybir

    if getattr(tile_mod.TileContext, "_wait_split_patched", False):
        return
    from concourse.tile import TileContext, ScopedClock

    orig_commit = TileContext._commit_instruction

    def commit_split(self, inst, lazy_reg_writes: bool = True):
        si = inst.sync_info
        if (
            si is not None
            and len(si.on_wait) > 1
            and inst.engine is not None
            and inst.engine != mybir.EngineType.Unassigned
        ):
            waits = list(si.on_wait)
            for wcond in waits[:-1]:
                nop = mybir.InstNoOp(
                    name=self.nc.get_next_instruction_name(),
                    engine=inst.engine,
                    sync_info=mybir.SyncInfo(on_wait=[wcond], on_update=[]),
                    bass_nofuse=True,
                )
                orig_commit(self, nop, lazy_reg_writes=False)
            si.on_wait = waits[-1:]
            inst.sync_info = si
        return orig_commit(self, inst, lazy_reg_writes)

    def drain_split(self, tick_clock, wait_clock):
        nc = self.nc
        carrier = nc.sync.drain()
        wait_clock.add_sem_waits(
            carrier.ins, ScopedClock({None: tick_clock.global_clock})
        )
        waits = list(carrier.ins.sync_info.on_wait)
        if len(waits) > 1:
            si = carrier.ins.sync_info
            si.on_wait = waits[:1]
            carrier.ins.sync_info = si
            for w in waits[1:]:
                extra = nc.sync.drain()
                extra.ins.sync_info = mybir.SyncInfo(on_wait=[w], on_update=[])
        nc.all_engine_barrier()
        assert self.sems is not None
        popped = nc._tile_sem_poison_stack.pop()
        assert popped is self._sem_poison
        nc.clear_and_free_semaphores(list(self.sems.allocated().values()))
        nc.all_engine_barrier()

    TileContext._commit_instruction = commit_split
    TileContext._drain_and_barrier = drain_split
    TileContext._wait_split_patched = True


# ---------------------------------------------------------------------------
# device kernel build
# ---------------------------------------------------------------------------

def _build_kernel():
    if "nc" in _BUILT:
        return _BUILT["nc"]

    import concourse.bass as bass
    import concourse.tile as tile
    from concourse import mybir

    _install_patches()

    f16 = mybir.dt.float16
    f32 = mybir.dt.float32
    Alu = mybir.AluOpType
    Act = mybir.ActivationFunctionType

    nc = bass.Bass()
    x16_d = nc.declare_dram_parameter("x16", [128, NG * T], f16, isOutput=False)
    wcomb_d = [
        nc.declare_dram_parameter(f"wcomb{i}", [SWS[i], ZWS[i]], f16,
                                  isOutput=False)
        for i in range(NSTREAM)
    ]
    ident_d = nc.declare_dram_parameter("ident", [128, 128], f16, isOutput=False)
    h4_d = nc.declare_dram_parameter("h4", [128, NG * 6], f16, isOutput=True)

    with tile.TileContext(nc) as tc:
        with (
            tc.tile_pool(name="persist", bufs=1) as persist,
            tc.tile_pool(name="work", bufs=2) as work,
            tc.tile_pool(name="st", bufs=2) as stp,
            tc.tile_pool(name="psum_tr", bufs=2, space="PSUM") as ptr2,
            tc.tile_pool(name="psum_tr1", bufs=1, space="PSUM") as ptr1,
            tc.tile_pool(name="psum_z", bufs=1, space="PSUM") as pz,
        ):
            x16 = persist.tile([128, NG * T], f16)
            wcomb = [
                persist.tile([SWS[i], ZWS[i]], f16, tag=f"wc{i}", name=f"wc{i}")
                for i in range(NSTREAM)
            ]
            ident = persist.tile([128, 128], f16)
            nc.sync.dma_start(x16[:], x16_d[:])
            for i in range(NSTREAM):
                nc.sync.dma_start(wcomb[i][:], wcomb_d[i][:])
            nc.sync.dma_start(ident[:], ident_d[:])

            S = []
            GD = []
            for si_ in range(NSTREAM):
                s_t = persist.tile([128, NSLOT * SWS[si_]], f16, tag=f"S{si_}", name=f"S{si_}")
                # GD holds [D(48) | tg(48) | tf(48) | ti(48) | to(48)]; the
                # gate regions are rewritten by tanh(z) every step while D
                # persists, letting (Q,R) fuse into one 96-wide op.
                gd_t = persist.tile([128, 5 * CWS[si_]], f16, tag=f"GD{si_}", name=f"GD{si_}")
                s2 = s_t.rearrange("p (s f) -> p s f", s=NSLOT)
                eng = [nc.vector, nc.gpsimd, nc.gpsimd][si_]
                # zero only the wavefront-edge slots; interior slots are
                # written (all state comps) before they are read.
                eng.memset(s2[:, 0:5, :], 0.0)
                eng.memset(s2[:, T:NSLOT, :], 0.0)
                eng.memset(gd_t[:, 0:CWS[si_]], 0.0)
                S.append(s_t)
                GD.append(gd_t)

            # x prefill: S[:, slot t, group g, comp 0] = x16[:, goff+g, t]
            x3 = x16.rearrange("p (g t) -> p t g", g=NG)
            for si_ in range(NSTREAM):
                goff = GOFF[si_]
                s4 = S[si_].rearrange(
                    "p (s g c) -> p s g c", s=NSLOT, g=GCS[si_], c=SLOT
                )
                eng = [nc.vector, nc.gpsimd, nc.gpsimd][si_]
                eng.tensor_copy(
                    s4[:, 0:T, :, 0], x3[:, 0:T, goff:goff + GCS[si_]]
                )

            s2v = [S[si_].rearrange("p (s f) -> p s f", s=NSLOT)
                   for si_ in range(NSTREAM)]
            s4v = [S[si_].rearrange("p (s g c) -> p s g c",
                                    s=NSLOT, g=GCS[si_], c=SLOT)
                   for si_ in range(NSTREAM)]
            gdv = [GD[si_].rearrange("p (x g m) -> p x g m",
                                     x=5, g=GCS[si_], m=12)
                   for si_ in range(NSTREAM)]
            cur = [{} for _ in range(NSTREAM)]

            def tr_(si_, s):
                pool = ptr2 if si_ < 2 else ptr1
                trp = pool.tile([SWS[si_], 128], f16, tag=f"tr{si_}", name=f"tr{si_}")
                nc.tensor.transpose(trp[:], s2v[si_][:, s, :], ident[:])
                cur[si_]["trp"] = trp

            def cp_(si_, s):
                st = stp.tile([SWS[si_], 128], f16, tag=f"st{si_}", name=f"st{si_}")
                nc.vector.tensor_copy(st[:], cur[si_]["trp"][:])
                cur[si_]["st"] = st

            def mm_(si_, s):
                z = pz.tile([128, ZWS[si_]], f32, tag=f"z{si_}", name=f"z{si_}")
                nc.tensor.matmul(
                    z[:], cur[si_]["st"][:], wcomb[si_][:],
                    start=True, stop=True
                )
                cur[si_]["z"] = z

            def tanhg_(si_, s):
                zv = cur[si_]["z"].rearrange(
                    "p (a g m) -> p a g m", a=4, g=GCS[si_], m=12
                )
                nc.scalar.activation(gdv[si_][:, 1:5], zv[:], Act.Tanh)

            def qr_(si_, s):
                # [Q|R] = ([tf|ti]+1) * [D|tg]  (one fused 96-wide op)
                qr_t = work.tile([128, 2 * CWS[si_]], f16, tag=f"qr{si_}", name=f"qr{si_}")
                qrv = qr_t.rearrange("p (x g m) -> p x g m", x=2, g=GCS[si_], m=12)
                nc.vector.scalar_tensor_tensor(
                    qrv, gdv[si_][:, 2:4], 1.0, gdv[si_][:, 0:2],
                    Alu.add, Alu.mult
                )
                cur[si_]["qr"] = qrv

            def dp_(si_, s):
                # D' = 0.5*Q + R (in place into GD[:, 0:48])
                qrv = cur[si_]["qr"]
                nc.vector.scalar_tensor_tensor(
                    gdv[si_][:, 0], qrv[:, 0], 0.5, qrv[:, 1],
                    Alu.mult, Alu.add
                )

            def tanhc_(si_, s):
                tcs = s4v[si_][:, s + 1, :, 13:25]
                nc.scalar.activation(tcs, gdv[si_][:, 0], Act.Tanh, scale=0.5)

            def pp_(si_, s):
                # P = to * tc -> next slot (bf16)
                nc.vector.tensor_tensor(
                    s4v[si_][:, s + 1, :, 1:13], gdv[si_][:, 4],
                    s4v[si_][:, s + 1, :, 13:25], Alu.mult
                )

            for s in range(NSTEP):
                for si_ in range(NSTREAM):
                    tr_(si_, s)
                    cp_(si_, s)
                    mm_(si_, s)
                    tanhg_(si_, s)
                    qr_(si_, s)
                    dp_(si_, s)
                    tanhc_(si_, s)
                    pp_(si_, s)

            # output: P4 and tc4 of the final slot
            h4r = h4_d.rearrange("p (g u) -> p g u", g=NG, u=6)
            for si_ in range(NSTREAM):
                goff = GOFF[si_]
                gc = GCS[si_]
                s4 = S[si_].rearrange(
                    "p (s g c) -> p s g c", s=NSLOT, g=gc, c=SLOT
                )
                nc.sync.dma_start(
                    h4r[:, goff:goff + gc, 0:3], s4[:, NSTEP, :, 10:13]
                )
                nc.sync.dma_start(
                    h4r[:, goff:goff + gc, 3:6], s4[:, NSTEP, :, 22:25]
                )

    _BUILT["nc"] = nc
    return nc


# ---------------------------------------------------------------------------
# entry point
# ---------------------------------------------------------------------------

def kernel(state, w1, u1, b1, w2, u2, b2, w3, u3, b3, w4, u4, b4, wd, bd,
           _want_results=False, _trace=False):
    state = np.asarray(state, np.float32)
    assert state.shape == (B, T), state.shape
    w = [np.asarray(a, np.float32) for a in (w1, w2, w3, w4)]
    u = [np.asarray(a, np.float32) for a in (u1, u2, u3, u4)]
    wd_ = np.asarray(wd, np.float32)
    bd_ = np.asarray(bd, np.float32)

    wc = _build_wcomb(w, u)
    # stream-level column order is (a, g, m): [tg-all | tf-all | ti-all |
    # to-all], so tanh(z) lands directly in GD
    wcombs = []
    for i in range(NSTREAM):
        gc, cw = GCS[i], CWS[i]
        wcb = np.zeros((SWS[i], ZWS[i]), F16)
        for g in range(gc):
            for a in range(4):
                wcb[
                    SLOT * g:SLOT * (g + 1),
                    cw * a + 12 * g:cw * a + 12 * (g + 1)
                ] = wc[:, 12 * a:12 * (a + 1)]
        wcombs.append(wcb)
    ident = np.eye(128, dtype=F16)
    # x16[core, p, g*T + t] = state[1024*core + 128*g + p, t]
    x16 = (
        state.reshape(N_CORES, NG, 128, T)
        .transpose(0, 2, 1, 3)
        .reshape(N_CORES, 128, NG * T)
        .astype(F16)
    )

    nc = _build_kernel()
    from concourse.bass_utils import run_bass_kernel_spmd

    in_maps = [
        {"x16": x16[c], "ident": ident,
         **{f"wcomb{i}": wcombs[i] for i in range(NSTREAM)}}
        for c in range(N_CORES)
    ]
    kw = {}
    if _trace:
        kw = dict(trace=True)
    res = run_bass_kernel_spmd(nc, in_maps, list(range(N_CORES)), **kw)

    # gather: h4[c] is (128, NG*6) bf16 = [P4 | tc4]; h = 0.5*(P4 + tc4)
    h = np.zeros((B, UNITS), np.float32)
    for c in range(N_CORES):
        hc = np.asarray(res.results[c]["h4"], np.float32).reshape(128, NG, 6)
        hf = 0.5 * (hc[:, :, 0:3] + hc[:, :, 3:6])
        # b = 1024c + 128g + p
        h[1024 * c:1024 * (c + 1)] = hf.transpose(1, 0, 2).reshape(1024, 3)
    out = (h @ wd_ + bd_)[:, 0][None, :].astype(np.float32)
    if _want_results:
        return out, res
    return out


# revision 24
# speedup vs baseline: 1.0012x; 1.0012x over previous
"""Trainium2 Bass kernel for nn_Brain_17789754540385.

Model: 4 stacked Keras LSTMs (units=3) over (B=8192, T=256) scalar sequences,
then Dense(3->1); output (1, B).

Strategy (pure data parallel, 8 cores, 1024 batch rows each):
- Batch-on-partitions layout: 1024 rows = 8 groups x 128 partitions per
  core, split into 3 independent superstreams of (3, 3, 2) groups. Each
  stream is a fully serial recurrence chain (~2.5us per wavefront step);
  the wall clock is max(chain latency, engine throughput), and 3 streams
  is the measured optimum: fewer streams leave engines idle inside the
  chain latency, more streams saturate the DVE with per-op fixed costs.
- Wavefront over s = 0..258: layer l computes timestep t = s - l. Biases
  are zero, so zero state is a fixed point and wavefront edges need no
  masking (only edge slots are zeroed; interior slots are written before
  read).
- State representation: each slot stores (x, P_l = to_l*tc_l, tc_l) per
  layer, since 2h = (to+1)*tc = P + tc; both P and tc rows carry half the
  h-weights in the combined matmul, so no (to+1) op is materialized.
- Per superstream, per step (one serial chain):
    PE : transpose the fp16 state slot (128, 25g) -> (25g, 128) PSUM
    VEC: copy PSUM -> SBUF (matmul stationary must be SBUF)
    PE : block-diagonal matmul vs combined weights -> z (128, 48g) f32
    ACT: GD[tg|tf|ti|to] = tanh(z)   (sigmoid folded: sig(x) =
         0.5 + 0.5*tanh(x/2), the 1/2 lives in the f,i,o weight columns)
    VEC: [Q|R] = ([tf|ti]+1) * [D|tg]  (one fused op; D stores 2c and
         sits in the same tile as the gates to make the operands
         contiguous; Q = 4fc, R = 2ig)
    VEC: D' = 0.5*Q + R               (= 2c', in place)
    ACT: tc = tanh(0.5*D) -> next slot
    VEC: P = to * tc      -> next slot
- All 16-bit tensors are fp16 (better mantissa than bf16; rel err 1.9e-3
  vs 3.9e-3). GPSIMD only does setup work: per-step ops on the Q7 are
  280-870ns and poison the chain.
- Final dense (3 -> 1) and batch gather run on host in fp32.
"""

import numpy as np
import ml_dtypes

F16 = np.float16

UNITS = 3
N_CORES = 8
B = 8192
T = 256
NG = 8            # batch groups of 128 per core
NSTREAM = 3       # superstreams
GCS = [3, 3, 2]   # groups per superstream
GOFF = [0, 3, 6]
SLOT = 25         # comps per group per slot: [x, P1..P4 (12), tc1..tc4 (12)]
SWS = [g * SLOT for g in GCS]   # slot width per superstream
ZWS = [g * 48 for g in GCS]     # gate width per superstream
CWS = [g * 12 for g in GCS]     # cell width per superstream
NSTEP = T + 3     # 259 wavefront steps
NSLOT = NSTEP + 1

_BUILT = {}


# ---------------------------------------------------------------------------
# host-side weight prep
# ---------------------------------------------------------------------------

def _build_wcomb(w, u):
    """Combined stationary-side weight matrix (25, 48) as fp64->bf16.

    Rows: 0 = x; 1+3l+v = P_l unit v; 13+3l+v = tc_l unit v.
    Cols: gate-major a-blocks of 12: [g, f, i, o] x (layer-major (l, v)).
    Keras gate order in w/u is i, f, g, o. h_l = 0.5*(P_l + tc_l), so both
    P_l and tc_l rows carry half the h_l weights. f,i,o columns are halved
    once more for the sigmoid-as-tanh fold.
    """
    kmap = [2, 1, 0, 3]  # target gate a=[g,f,i,o] -> keras col block
    wcomb = np.zeros((SLOT, 48), np.float64)
    for l in range(4):
        wl = np.asarray(w[l], np.float64)   # (in_dim, 12) keras order
        ul = np.asarray(u[l], np.float64)   # (3, 12)
        for a in range(4):
            gs = 1.0 if a == 0 else 0.5
            for v in range(3):
                col = a * 12 + l * 3 + v
                kc = kmap[a] * 3 + v
                # recurrent: h_l rows (P_l and tc_l, half weight each)
                for vv in range(3):
                    wcomb[1 + 3 * l + vv, col] = ul[vv, kc] * gs * 0.5
                    wcomb[13 + 3 * l + vv, col] = ul[vv, kc] * gs * 0.5
                # input: x for layer 0, h_{l-1} rows otherwise
                if l == 0:
                    wcomb[0, col] = wl[0, kc] * gs
                else:
                    for vv in range(3):
                        wcomb[1 + 3 * (l - 1) + vv, col] = (
                            wl[vv, kc] * gs * 0.5
                        )
                        wcomb[13 + 3 * (l - 1) + vv, col] = (
                            wl[vv, kc] * gs * 0.5
                        )
    return wcomb.astype(F16)


# ---------------------------------------------------------------------------
# workarounds: this walrus build allows at most ONE sem wait per instruction
# ---------------------------------------------------------------------------

def _install_patches():
    import concourse.tile as tile_mod
    from concourse import mybir

    if getattr(tile_mod.TileContext, "_wait_split_patched", False):
        return
    from concourse.tile import TileContext, ScopedClock

    orig_commit = TileContext._commit_instruction

    def commit_split(self, inst, lazy_reg_writes: bool = True):
        si = inst.sync_info
        if (
            si is not None
            and len(si.on_wait) > 1
            and inst.engine is not None
            and inst.engine != mybir.EngineType.Unassigned
        ):
            waits = list(si.on_wait)
            for wcond in waits[:-1]:
                nop = mybir.InstNoOp(
                    name=self.nc.get_next_instruction_name(),
                    engine=inst.engine,
                    sync_info=mybir.SyncInfo(on_wait=[wcond], on_update=[]),
                    bass_nofuse=True,
                )
                orig_commit(self, nop, lazy_reg_writes=False)
            si.on_wait = waits[-1:]
            inst.sync_info = si
        return orig_commit(self, inst, lazy_reg_writes)

    def drain_split(self, tick_clock, wait_clock):
        nc = self.nc
        carrier = nc.sync.drain()
        wait_clock.add_sem_waits(
            carrier.ins, ScopedClock({None: tick_clock.global_clock})
        )
        waits = list(carrier.ins.sync_info.on_wait)
        if len(waits) > 1:
            si = carrier.ins.sync_info
            si.on_wait = waits[:1]
            carrier.ins.sync_info = si
            for w in waits[1:]:
                extra = nc.sync.drain()
                extra.ins.sync_info = mybir.SyncInfo(on_wait=[w], on_update=[])
        nc.all_engine_barrier()
        assert self.sems is not None
        popped = nc._tile_sem_poison_stack.pop()
        assert popped is self._sem_poison
        nc.clear_and_free_semaphores(list(self.sems.allocated().values()))
        nc.all_engine_barrier()

    TileContext._commit_instruction = commit_split
    TileContext._drain_and_barrier = drain_split
    TileContext._wait_split_patched = True


# ---------------------------------------------------------------------------
# device kernel build
# ---------------------------------------------------------------------------

def _build_kernel():
    if "nc" in _BUILT:
        return _BUILT["nc"]

    import concourse.bass as bass
    import concourse.tile as tile
    from concourse import mybir

    _install_patches()

    f16 = mybir.dt.float16
    f32 = mybir.dt.float32
    Alu = mybir.AluOpType
    Act = mybir.ActivationFunctionType

    nc = bass.Bass()
    x16_d = nc.declare_dram_parameter("x16", [128, NG * T], f16, isOutput=False)
    wcomb_d = [
        nc.declare_dram_parameter(f"wcomb{i}", [SWS[i], ZWS[i]], f16,
                                  isOutput=False)
        for i in range(NSTREAM)
    ]
    ident_d = nc.declare_dram_parameter("ident", [128, 128], f16, isOutput=False)
    h4_d = nc.declare_dram_parameter("h4", [128, NG * 6], f16, isOutput=True)

    with tile.TileContext(nc) as tc:
        with (
            tc.tile_pool(name="persist", bufs=1) as persist,
            tc.tile_pool(name="work", bufs=3) as work,
            tc.tile_pool(name="st", bufs=3) as stp,
            tc.tile_pool(name="psum_tr", bufs=2, space="PSUM") as ptr2,
            tc.tile_pool(name="psum_tr1", bufs=1, space="PSUM") as ptr1,
            tc.tile_pool(name="psum_z", bufs=1, space="PSUM") as pz,
        ):
            x16 = persist.tile([128, NG * T], f16)
            wcomb = [
                persist.tile([SWS[i], ZWS[i]], f16, tag=f"wc{i}", name=f"wc{i}")
                for i in range(NSTREAM)
            ]
            ident = persist.tile([128, 128], f16)
            nc.sync.dma_start(x16[:], x16_d[:])
            for i in range(NSTREAM):
                nc.sync.dma_start(wcomb[i][:], wcomb_d[i][:])
            nc.sync.dma_start(ident[:], ident_d[:])

            S = []
            GD = []
            for si_ in range(NSTREAM):
                s_t = persist.tile([128, NSLOT * SWS[si_]], f16, tag=f"S{si_}", name=f"S{si_}")
                # GD holds [D(48) | tg(48) | tf(48) | ti(48) | to(48)]; the
                # gate regions are rewritten by tanh(z) every step while D
                # persists, letting (Q,R) fuse into one 96-wide op.
                gd_t = persist.tile([128, 5 * CWS[si_]], f16, tag=f"GD{si_}", name=f"GD{si_}")
                s2 = s_t.rearrange("p (s f) -> p s f", s=NSLOT)
                eng = [nc.vector, nc.gpsimd, nc.gpsimd][si_]
                # zero only the wavefront-edge slots; interior slots are
                # written (all state comps) before they are read.
                eng.memset(s2[:, 0:5, :], 0.0)
                eng.memset(s2[:, T:NSLOT, :], 0.0)
                eng.memset(gd_t[:, 0:CWS[si_]], 0.0)
                S.append(s_t)
                GD.append(gd_t)

            # x prefill: S[:, slot t, group g, comp 0] = x16[:, goff+g, t]
            x3 = x16.rearrange("p (g t) -> p t g", g=NG)
            for si_ in range(NSTREAM):
                goff = GOFF[si_]
                s4 = S[si_].rearrange(
                    "p (s g c) -> p s g c", s=NSLOT, g=GCS[si_], c=SLOT
                )
                eng = [nc.vector, nc.gpsimd, nc.gpsimd][si_]
                eng.tensor_copy(
                    s4[:, 0:T, :, 0], x3[:, 0:T, goff:goff + GCS[si_]]
                )

            s2v = [S[si_].rearrange("p (s f) -> p s f", s=NSLOT)
                   for si_ in range(NSTREAM)]
            s4v = [S[si_].rearrange("p (s g c) -> p s g c",
                                    s=NSLOT, g=GCS[si_], c=SLOT)
                   for si_ in range(NSTREAM)]
            gdv = [GD[si_].rearrange("p (x g m) -> p x g m",
                                     x=5, g=GCS[si_], m=12)
                   for si_ in range(NSTREAM)]
            cur = [{} for _ in range(NSTREAM)]

            def tr_(si_, s):
                pool = ptr2 if si_ < 2 else ptr1
                trp = pool.tile([SWS[si_], 128], f16, tag=f"tr{si_}", name=f"tr{si_}")
                nc.tensor.transpose(trp[:], s2v[si_][:, s, :], ident[:])
                cur[si_]["trp"] = trp

            def cp_(si_, s):
                st = stp.tile([SWS[si_], 128], f16, tag=f"st{si_}", name=f"st{si_}")
                nc.vector.tensor_copy(st[:], cur[si_]["trp"][:])
                cur[si_]["st"] = st

            def mm_(si_, s):
                z = pz.tile([128, ZWS[si_]], f32, tag=f"z{si_}", name=f"z{si_}")
                nc.tensor.matmul(
                    z[:], cur[si_]["st"][:], wcomb[si_][:],
                    start=True, stop=True
                )
                cur[si_]["z"] = z

            def tanhg_(si_, s):
                zv = cur[si_]["z"].rearrange(
                    "p (a g m) -> p a g m", a=4, g=GCS[si_], m=12
                )
                nc.scalar.activation(gdv[si_][:, 1:5], zv[:], Act.Tanh)

            def qr_(si_, s):
                # [Q|R] = ([tf|ti]+1) * [D|tg]  (one fused 96-wide op)
                qr_t = work.tile([128, 2 * CWS[si_]], f16, tag=f"qr{si_}", name=f"qr{si_}")
                qrv = qr_t.rearrange("p (x g m) -> p x g m", x=2, g=GCS[si_], m=12)
                nc.vector.scalar_tensor_tensor(
                    qrv, gdv[si_][:, 2:4], 1.0, gdv[si_][:, 0:2],
                    Alu.add, Alu.mult
                )
                cur[si_]["qr"] = qrv

            def dp_(si_, s):
                # D' = 0.5*Q + R (in place into GD[:, 0:48])
                qrv = cur[si_]["qr"]
                nc.vector.scalar_tensor_tensor(
                    gdv[si_][:, 0], qrv[:, 0], 0.5, qrv[:, 1],
                    Alu.mult, Alu.add
                )

            def tanhc_(si_, s):
                tcs = s4v[si_][:, s + 1, :, 13:25]
                nc.scalar.activation(tcs, gdv[si_][:, 0], Act.Tanh, scale=0.5)

            def pp_(si_, s):
                # P = to * tc -> next slot (bf16)
                nc.vector.tensor_tensor(
                    s4v[si_][:, s + 1, :, 1:13], gdv[si_][:, 4],
                    s4v[si_][:, s + 1, :, 13:25], Alu.mult
                )

            for s in range(NSTEP):
                for si_ in range(NSTREAM):
                    tr_(si_, s)
                    cp_(si_, s)
                    mm_(si_, s)
                    tanhg_(si_, s)
                    qr_(si_, s)
                    dp_(si_, s)
                    tanhc_(si_, s)
                    pp_(si_, s)

            # output: P4 and tc4 of the final slot
            h4r = h4_d.rearrange("p (g u) -> p g u", g=NG, u=6)
            for si_ in range(NSTREAM):
                goff = GOFF[si_]
                gc = GCS[si_]
                s4 = S[si_].rearrange(
                    "p (s g c) -> p s g c", s=NSLOT, g=gc, c=SLOT
                )
                nc.sync.dma_start(
                    h4r[:, goff:goff + gc, 0:3], s4[:, NSTEP, :, 10:13]
                )
                nc.sync.dma_start(
                    h4r[:, goff:goff + gc, 3:6], s4[:, NSTEP, :, 22:25]
                )

    _BUILT["nc"] = nc
    return nc


# ---------------------------------------------------------------------------
# entry point
# ---------------------------------------------------------------------------

def kernel(state, w1, u1, b1, w2, u2, b2, w3, u3, b3, w4, u4, b4, wd, bd,
           _want_results=False, _trace=False):
    state = np.asarray(state, np.float32)
    assert state.shape == (B, T), state.shape
    w = [np.asarray(a, np.float32) for a in (w1, w2, w3, w4)]
    u = [np.asarray(a, np.float32) for a in (u1, u2, u3, u4)]
    wd_ = np.asarray(wd, np.float32)
    bd_ = np.asarray(bd, np.float32)

    wc = _build_wcomb(w, u)
    # stream-level column order is (a, g, m): [tg-all | tf-all | ti-all |
    # to-all], so tanh(z) lands directly in GD
    wcombs = []
    for i in range(NSTREAM):
        gc, cw = GCS[i], CWS[i]
        wcb = np.zeros((SWS[i], ZWS[i]), F16)
        for g in range(gc):
            for a in range(4):
                wcb[
                    SLOT * g:SLOT * (g + 1),
                    cw * a + 12 * g:cw * a + 12 * (g + 1)
                ] = wc[:, 12 * a:12 * (a + 1)]
        wcombs.append(wcb)
    ident = np.eye(128, dtype=F16)
    # x16[core, p, g*T + t] = state[1024*core + 128*g + p, t]
    x16 = (
        state.reshape(N_CORES, NG, 128, T)
        .transpose(0, 2, 1, 3)
        .reshape(N_CORES, 128, NG * T)
        .astype(F16)
    )

    nc = _build_kernel()
    from concourse.bass_utils import run_bass_kernel_spmd

    in_maps = [
        {"x16": x16[c], "ident": ident,
         **{f"wcomb{i}": wcombs[i] for i in range(NSTREAM)}}
        for c in range(N_CORES)
    ]
    kw = {}
    if _trace:
        kw = dict(trace=True)
    res = run_bass_kernel_spmd(nc, in_maps, list(range(N_CORES)), **kw)

    # gather: h4[c] is (128, NG*6) bf16 = [P4 | tc4]; h = 0.5*(P4 + tc4)
    h = np.zeros((B, UNITS), np.float32)
    for c in range(N_CORES):
        hc = np.asarray(res.results[c]["h4"], np.float32).reshape(128, NG, 6)
        hf = 0.5 * (hc[:, :, 0:3] + hc[:, :, 3:6])
        # b = 1024c + 128g + p
        h[1024 * c:1024 * (c + 1)] = hf.transpose(1, 0, 2).reshape(1024, 3)
    out = (h @ wd_ + bd_)[:, 0][None, :].astype(np.float32)
    if _want_results:
        return out, res
    return out
